# revision 1
# baseline (speedup 1.0000x reference)
"""Trainium2 Bass kernel: 16-head attention with RoPE (dense_transformer).

Sharding: tensor-parallel over heads. 8 cores x 2 heads each.
Each core: Wq/Wk/Wv column slice [1024,128], Wo row slice [128,1024],
full input; computes its heads' attention + partial output projection.
Host sums the 8 partial outputs (row-parallel Wo reduction) and adds bo.

Device layout is "transposed": Q^T/K^T/ctx^T are kept as [dim, seq] with
the head dim on SBUF partitions, so Q^T = Wq^T @ X^T comes straight out
of the PE, scores^T = K^T.T @ Q^T needs no transposes, and the softmax
denominator falls out of an extra ones-column appended to V.

v4 schedule: the kernel is paced by the softmax exp (only the ACT
engine has an exp LUT; ACTIVATE costs ~(FD+313)ns so 16.8M elements
cost 142us at FD=1024). Everything else hides in its shadow:
 - xt arrives as 8 channel-major chunks (8KB DMA lines); batch-0's Q/K
   chains interleave per chunk, so rope(b0) starts ~26us and the first
   exp fires ~34us. Batch-1's Q/K chains + rope run as pump units
   inside the first attention blocks' exp shadow.
 - the two heads' score matmuls issue back-to-back as concurrent PE
   row-group tiles (0,0)/(64,0) into the two banks of ONE [128,1024]
   PSUM tile, so a single FD=1024 exp covers both heads.
 - each block's ctx chains ride the NEXT block's exp shadow, as do the
   V-projection chains and batch-0's output projection ("pump" units).
 - softmax division: reciprocal on the den row, gpsimd
   partition_broadcast (no DRAM bounce), one multiply per head.
 - output tiles leave via [128,4,512] quad-DMAs on alternating queues
   (per-tile DMAs pay ~2us completion latency each).
"""

import sys

if "/opt/trn_rl_repo" not in sys.path:
    sys.path.insert(0, "/opt/trn_rl_repo")

from collections import deque

import numpy as np
import ml_dtypes

B = 2
S = 2048
NS = B * S  # 4096
D = 1024
H = 16
DK = 64
NCORES = 8
HPC = H // NCORES  # heads per core = 2
DPC = HPC * DK  # model dims per core = 128

_cache = {}


def _build_nc():
    import concourse.bass as bass
    import concourse.tile as tile
    import concourse.mybir as mybir
    from concourse import bacc

    fp32 = mybir.dt.float32
    bf16 = mybir.dt.bfloat16
    Exp = mybir.ActivationFunctionType.Exp

    nc = bacc.Bacc("TRN2", debug=False, num_devices=NCORES)

    xt = nc.dram_tensor("xt", [D, NS], bf16, kind="ExternalInput").ap()
    wq = nc.dram_tensor("wq", [128, 8 * 128], bf16, kind="ExternalInput").ap()
    wk = nc.dram_tensor("wk", [128, 8 * 128], bf16, kind="ExternalInput").ap()
    wv = nc.dram_tensor("wv", [128, 8 * 128], bf16, kind="ExternalInput").ap()
    wo = nc.dram_tensor("wo", [DPC, D], bf16, kind="ExternalInput").ap()
    bq = nc.dram_tensor("bq", [DPC, 1], fp32, kind="ExternalInput").ap()
    bk = nc.dram_tensor("bk", [DPC, 1], fp32, kind="ExternalInput").ap()
    bv = nc.dram_tensor("bv", [1, DPC], bf16, kind="ExternalInput").ap()
    cos_d = nc.dram_tensor("cos", [128, S], bf16, kind="ExternalInput").ap()
    sin_d = nc.dram_tensor("sin", [128, S], bf16, kind="ExternalInput").ap()
    out_d = nc.dram_tensor("out", [D, NS], bf16, kind="ExternalOutput").ap()

    with tile.TileContext(nc) as tc:
        with (
            tc.tile_pool(name="persist", bufs=1) as persist,
            tc.tile_pool(name="qkv_sb", bufs=1) as qkv_sb,
            tc.tile_pool(name="qkv_tmp", bufs=1) as qkv_tmp,
            tc.tile_pool(name="att_sb", bufs=1) as att_sb,
            tc.tile_pool(name="op_sb", bufs=2) as op_sb,
            tc.tile_pool(name="dram", bufs=1, space="DRAM") as dram,
        ):
            qrot = persist.tile([128, NS], bf16, tag="qrot")
            krot = persist.tile([128, NS], bf16, tag="krot")
            # v_sb[:, tt, 65h : 65h+64] = V rows tt*128.. for head h,
            # v_sb[:, tt, 65h+64] = 1.0 (denominator column)
            v_sb = persist.tile([128, 32, 2 * (DK + 1)], bf16, tag="v")
            ctxT = persist.tile([128, NS], bf16, tag="ctxT")
            wo_sb = persist.tile([128, 8, 128], bf16, tag="wo")
            # den chunks land on 32-aligned rows: row 32*(b*2+h), cols
            # st_i*512.. (TensorCopy across partitions needs 32-aligned
            # bases; tensor_tensor ops need exact matches)
            den_big = persist.tile([128, 512], bf16, tag="den_big")
            dn128 = persist.tile([128, 4, 16], bf16, tag="dn128")
            rc128 = persist.tile([128, 4, 16], bf16, tag="rc128")
            den_dram = dram.tile([4 * S], bf16, tag="den_dram")
            rec_dram = dram.tile([4 * S], bf16, tag="rec_dram")

            cos_sb = qkv_sb.tile([128, S], bf16, tag="cos")
            sin_sb = qkv_sb.tile([128, S], bf16, tag="sin")
            wq_sb = qkv_sb.tile([128, 8, 128], bf16, tag="wq")
            wk_sb = qkv_sb.tile([128, 8, 128], bf16, tag="wk")
            wv_sb = qkv_sb.tile([128, 8, 128], bf16, tag="wv")
            bq_sb = qkv_sb.tile([128, 1], fp32, tag="bq")
            bk_sb = qkv_sb.tile([128, 1], fp32, tag="bk")
            bvb = qkv_sb.tile([128, 128], bf16, tag="bvb")
            xt_sb = qkv_sb.tile([128, 8, NS], bf16, tag="xt")
            xt_r = xt.rearrange("(c p) s -> p c s", p=128)
            out_r = out_d.rearrange("(j p) s -> p j s", p=128)

            # DMA plan: 3 queues. sync: wq + even chunks; gpsimd: wk + odd
            # chunks + wv/cos/sin; scalar(ACT, idle now): small tail.
            nc.sync.dma_start(wq_sb[:], wq.rearrange("p (c m) -> p c m", m=128))
            nc.gpsimd.dma_start(wk_sb[:], wk.rearrange("p (c m) -> p c m", m=128))
            for c in range(8):
                eng = nc.sync if c % 2 == 0 else nc.gpsimd
                eng.dma_start(xt_sb[:, c : c + 1, :], xt_r[:, c : c + 1, :])
            nc.scalar.dma_start(bq_sb[:], bq)
            nc.scalar.dma_start(bk_sb[:], bk)
            nc.scalar.dma_start(cos_sb[:], cos_d)
            nc.scalar.dma_start(sin_sb[:], sin_d)
            nc.scalar.dma_start(bvb[:], bv.to_broadcast((128, 128)))
            nc.scalar.dma_start(wv_sb[:], wv.rearrange("p (c m) -> p c m", m=128))
            nc.scalar.dma_start(wo_sb[:], wo.rearrange("p (j m) -> p j m", m=128))

            def rope(plain, rot_half):
                # rot = plain*cos + swap(plain)*sin, sin-mul in place
                swap = qkv_tmp.tile(
                    [128, S], bf16, tag="swap", bufs=2, name="swap"
                )
                for g in (0, 64):
                    nc.sync.dma_start(
                        swap[g : g + 32, :], plain[g + 32 : g + 64, :]
                    )
                    nc.sync.dma_start(
                        swap[g + 32 : g + 64, :], plain[g : g + 32, :]
                    )
                nc.vector.tensor_mul(rot_half, plain[:], cos_sb[:])
                nc.vector.tensor_mul(swap[:], swap[:], sin_sb[:])
                nc.vector.tensor_add(rot_half, rot_half, swap[:])

            # -------- Phase 1: batch-0 Q/K projections + RoPE -------------
            # Q and K chains interleave per xt chunk so both track the DMA;
            # done ~26us in. The batch-1 halves run later as pump units.
            plain_q0 = qkv_tmp.tile([128, S], bf16, tag="plain", name="plain_q0")
            plain_k0 = qkv_tmp.tile([128, S], bf16, tag="plain2", name="plain_k0")
            with tc.tile_pool(name="qkv_ps", bufs=1, space="PSUM") as qkv_ps:
                ps_q = [
                    qkv_ps.tile([128, 512], fp32, tag=f"b{i}", name=f"psq{i}")
                    for i in range(4)
                ]
                ps_k = [
                    qkv_ps.tile([128, 512], fp32, tag=f"b{4 + i}", name=f"psk{i}")
                    for i in range(4)
                ]
                for ch in range(8):
                    for st in range(4):
                        nc.tensor.matmul(
                            ps_q[st][:],
                            wq_sb[:, ch, :],
                            xt_sb[:, ch, st * 512 : (st + 1) * 512],
                            start=(ch == 0),
                            stop=(ch == 7),
                        )
                    for st in range(4):
                        nc.tensor.matmul(
                            ps_k[st][:],
                            wk_sb[:, ch, :],
                            xt_sb[:, ch, st * 512 : (st + 1) * 512],
                            start=(ch == 0),
                            stop=(ch == 7),
                        )
                Ident = mybir.ActivationFunctionType.Identity
                for st in range(4):
                    nc.scalar.activation(
                        plain_q0[:, st * 512 : (st + 1) * 512],
                        ps_q[st][:],
                        Ident,
                        bias=bq_sb[:],
                    )
                for st in range(4):
                    nc.scalar.activation(
                        plain_k0[:, st * 512 : (st + 1) * 512],
                        ps_k[st][:],
                        Ident,
                        bias=bk_sb[:],
                    )
                rope(plain_q0, qrot[:, 0:S])
                rope(plain_k0, krot[:, 0:S])

                ones_ap = v_sb[:].rearrange("p t (h x) -> p t h x", x=DK + 1)[
                    :, :, :, DK
                ]
                nc.vector.memset(ones_ap, 1.0)

            # ---------------- Phase 2: attention ------------------------
            with (
                tc.tile_pool(name="sc_ps", bufs=2, space="PSUM") as sc_ps,
                tc.tile_pool(name="ctx_ps", bufs=2, space="PSUM") as ctx_ps,
                tc.tile_pool(name="op_ps", bufs=2, space="PSUM") as op_ps,
            ):
                work = deque()

                def pump(n):
                    for _ in range(n):
                        if work:
                            work.popleft()()

                def v_chain(tt):
                    def unit():
                        psv = op_ps.tile(
                            [128, 128], fp32, tag="op", name=f"psv{tt}"
                        )
                        for ch in range(8):
                            nc.tensor.matmul(
                                psv[:],
                                xt_sb[:, ch, tt * 128 : (tt + 1) * 128],
                                wv_sb[:, ch, :],
                                start=(ch == 0),
                                stop=(ch == 7),
                            )
                        dst = v_sb[:, tt].rearrange("p (h x) -> p h x", h=2)[
                            :, :, 0:DK
                        ]
                        nc.vector.tensor_add(dst, psv[:], bvb[:])

                    return unit

                def qk_b1_chain(w_sb, b_sb, plain, st):
                    def unit():
                        psq = op_ps.tile(
                            [128, 512], fp32, tag="op", name=f"qk1_{st}"
                        )
                        for ch in range(8):
                            nc.tensor.matmul(
                                psq[:],
                                w_sb[:, ch, :],
                                xt_sb[:, ch, st * 512 : (st + 1) * 512],
                                start=(ch == 0),
                                stop=(ch == 7),
                            )
                        nc.vector.tensor_scalar_add(
                            plain[:, (st - 4) * 512 :][:, 0:512], psq[:], b_sb[:]
                        )

                    return unit

                _ob_cycle = [("ob", op_sb), ("plain", qkv_tmp), ("ob", op_sb),
                             ("plain2", qkv_tmp)]

                def op_quad(st, j, engs, po_pools=None):
                    # 2 out-proj tiles (oc = 2j, 2j+1) -> one 128KB DMA;
                    # 4-deep staging rotation (op_sb + dead rope slots)
                    # hides the ~2us DMA completion latency
                    def unit():
                        tagname, pool = _ob_cycle[(st * 4 + j) % 4]
                        ob = pool.tile(
                            [128, 2, 512], bf16, tag=tagname, bufs=None,
                            name=f"ob{st}_{j}",
                        )
                        for k in range(2):
                            oc = j * 2 + k
                            if po_pools is None:
                                po = op_ps.tile(
                                    [128, 512], fp32, tag="op",
                                    name=f"po{st}_{oc}",
                                )
                            else:
                                pool, ptag = po_pools[oc % 2]
                                po = pool.tile(
                                    [128, 512], fp32, tag=ptag, bufs=2,
                                    name=f"po{st}_{oc}",
                                )
                            nc.tensor.matmul(
                                po[:],
                                wo_sb[:, oc, :],
                                ctxT[:, st * 512 : (st + 1) * 512],
                                start=True,
                                stop=True,
                            )
                            engs[k % len(engs)](ob[:, k, :], po[:])
                        dq = nc.sync if (st + j) % 2 == 0 else nc.gpsimd
                        dq.dma_start(
                            out_r[:, j * 2 : j * 2 + 2, st * 512 : (st + 1) * 512],
                            ob[:],
                        )

                    return unit

                def div_unit(b, h, dq):
                    # reciprocal of the den row, DRAM bounce to broadcast
                    # along partitions, scale this head's ctxT rows
                    def unit():
                        pi = b * HPC + h
                        hh = h * DK
                        with nc.allow_low_precision(
                            reason="bf16 softmax reciprocal within tolerance"
                        ):
                            nc.vector.reciprocal(rc128[:, pi, :], dn128[:, pi, :])
                        dq.dma_start(
                            rec_dram[pi * S : (pi + 1) * S].rearrange(
                                "(p c) -> p c", p=128
                            ),
                            rc128[:, pi, :],
                        )
                        R = att_sb.tile(
                            [128, S], bf16, tag="R", bufs=2, name=f"R{pi}"
                        )
                        dq.dma_start(
                            R[hh : hh + DK, :],
                            rec_dram[pi * S : (pi + 1) * S]
                            .rearrange("(o s) -> o s", o=1)
                            .to_broadcast((DK, S)),
                        )
                        sl = slice(b * S, (b + 1) * S)
                        nc.vector.tensor_mul(
                            ctxT[hh : hh + DK, sl],
                            ctxT[hh : hh + DK, sl],
                            R[hh : hh + DK, :],
                        )

                    return unit

                def ctx_units(b, sh, si, expS):
                    # two 16-matmul ctx chains (head 0 / head 1) for the
                    # 512-col stripe st_i, as 8 pump units of 4 MMs + one
                    # evacuation unit. Chains ride the NEXT block's exp.
                    st_i = sh * 2 + si
                    pcs = {}

                    def chain_quarter(h, q):
                        def unit():
                            if q == 0:
                                pcs[h] = ctx_ps.tile(
                                    [DK + 1, 512],
                                    fp32,
                                    tag="pc",
                                    name=f"pc{b}{st_i}{h}",
                                )
                            for tt in range(q * 4, q * 4 + 4):
                                nc.tensor.matmul(
                                    pcs[h][:],
                                    v_sb[
                                        :,
                                        b * 16 + tt,
                                        h * (DK + 1) : (h + 1) * (DK + 1),
                                    ],
                                    expS[:, tt, h * 512 : (h + 1) * 512],
                                    start=(tt == 0),
                                    stop=(tt == 15),
                                )

                        return unit

                    def evacuate():
                        ds0 = b * S + st_i * 512
                        for h in range(HPC):
                            pc = pcs[h]
                            pi = b * HPC + h
                            if h == 0:
                                nc.vector.tensor_copy(
                                    ctxT[0:DK, ds0 : ds0 + 512], pc[0:DK, :]
                                )
                            else:
                                stg = att_sb.tile(
                                    [DK, 512], bf16, tag="stg", bufs=1, name="stg"
                                )
                                nc.vector.tensor_copy(stg[:], pc[0:DK, :])
                                nc.vector.stream_shuffle(
                                    ctxT[DK : 2 * DK, ds0 : ds0 + 512],
                                    stg[:],
                                    mask=list(range(32)),
                                )
                            nc.vector.tensor_copy(
                                den_big[32 * pi : 32 * pi + 1, :],
                                pc[DK : DK + 1, :],
                            )
                            d0 = pi * S + st_i * 512
                            nc.gpsimd.dma_start(
                                den_dram[d0 : d0 + 512].rearrange(
                                    "(o c) -> o c", o=1
                                ),
                                den_big[32 * pi : 32 * pi + 1, :],
                            )
                            nc.gpsimd.dma_start(
                                dn128[st_i * 32 : (st_i + 1) * 32, pi, :],
                                den_dram[d0 : d0 + 512].rearrange(
                                    "(p c) -> p c", p=32
                                ),
                            )

                    units = []
                    for q in range(4):
                        units.append(chain_quarter(0, q))
                        units.append(chain_quarter(1, q))
                    units.append(evacuate)
                    return units

                def attn_block(b, sh, si, budget):
                    # both heads' scores into the two banks of one
                    # [128,1024] PSUM tile (concurrent row-group tiles
                    # (0,0)/(64,0)); ONE FD=1024 exp covers both heads.
                    # expS[:, tt, 0:512]=head0, [:, tt, 512:]=head1.
                    expS = att_sb.tile(
                        [128, 16, 1024], bf16, tag="expS", bufs=2,
                        name=f"eS{b}{sh}{si}",
                    )
                    s0 = b * S + sh * 1024 + si * 512
                    for tt in range(16):
                        pump(budget)
                        tb = slice(b * S + tt * 128, b * S + (tt + 1) * 128)
                        ps = sc_ps.tile([128, 1024], fp32, tag="sc", name="psAB")
                        nc.tensor.matmul(
                            ps[:, 0:512], krot[0:DK, tb], qrot[0:DK, s0 : s0 + 512],
                            start=True, stop=True,
                        )
                        nc.tensor.matmul(
                            ps[:, 512:1024],
                            krot[DK:128, tb],
                            qrot[DK:128, s0 : s0 + 512],
                            start=True, stop=True,
                        )
                        nc.scalar.activation(expS[:, tt, :], ps[:], Exp, scale=0.125)
                    return expS

                # pump inventory: V(b0) chains first (xt fully landed by the
                # first block), then batch-1 Q/K chains + their rope, then
                # V(b1); ctx units always jump the queue (extendleft).
                for tt in range(16):
                    work.append(v_chain(tt))
                plain_q1 = qkv_tmp.tile(
                    [128, S], bf16, tag="plain", name="plain_q1"
                )
                plain_k1 = qkv_tmp.tile(
                    [128, S], bf16, tag="plain2", name="plain_k1"
                )
                for st in range(4, 8):
                    work.append(qk_b1_chain(wq_sb, bq_sb, plain_q1, st))
                work.append(lambda: rope(plain_q1, qrot[:, S:NS]))
                for st in range(4, 8):
                    work.append(qk_b1_chain(wk_sb, bk_sb, plain_k1, st))
                work.append(lambda: rope(plain_k1, krot[:, S:NS]))

                blocks = [
                    (b, sh, si)
                    for b in range(B)
                    for sh in range(2)
                    for si in range(2)
                ]
                prev = None
                for bi, (b, sh, si) in enumerate(blocks):
                    if bi == 1:
                        for tt in range(16, 32):
                            work.append(v_chain(tt))
                    if bi == 5:
                        work.append(div_unit(0, 0, nc.gpsimd))
                        work.append(div_unit(0, 1, nc.gpsimd))
                        engs = [nc.vector.tensor_copy]
                        for st in range(4):
                            for j in range(4):
                                work.append(op_quad(st, j, engs))
                    if prev is not None:
                        work.extendleft(reversed(ctx_units(*prev)))
                    expS = attn_block(b, sh, si, budget=1 if bi == 0 else 2)
                    prev = (b, sh, si, expS)

                # drain: last block's ctx, remaining pump work, b1 divs,
                # b1 out-projection
                for u in ctx_units(*prev):
                    u()
                while work:
                    work.popleft()()
                div_unit(1, 0, nc.gpsimd)()
                div_unit(1, 1, nc.sync)()
                engs = [nc.vector.tensor_copy, nc.scalar.copy]
                po_pools = [(op_ps, "op"), (sc_ps, "sc")]
                for st in range(4, 8):
                    for j in range(4):
                        op_quad(st, j, engs, po_pools)()

    nc.compile()
    return nc


def _rope_tables():
    pos = np.arange(S, dtype=np.float64)
    inv_freq = np.exp(np.arange(0, DK, 2, dtype=np.float64) * (-np.log(10000.0) / DK))
    ang = pos[:, None] * inv_freq[None, :]  # [S, 32]
    cos_t = np.empty((128, S), dtype=np.float32)
    sin_t = np.empty((128, S), dtype=np.float32)
    c = np.cos(ang).astype(np.float32).T  # [32, S]
    s = np.sin(ang).astype(np.float32).T
    for blk in range(4):
        cos_t[blk * 32 : (blk + 1) * 32] = c
        sign = -1.0 if blk % 2 == 0 else 1.0
        sin_t[blk * 32 : (blk + 1) * 32] = sign * s
    return cos_t, sin_t


def _prep_w(w):
    # [1024, 128] column slice -> [128, 8*128] with the 1024-dim split into
    # 8 chunks of 128 on the partition axis (contiguous 2KB DMA lines)
    bf = ml_dtypes.bfloat16
    return np.ascontiguousarray(
        np.asarray(w, dtype=np.float32)
        .reshape(8, 128, 128)
        .transpose(1, 0, 2)
        .reshape(128, 8 * 128)
    ).astype(bf)


def _prep_inputs(inputs, Wq, bq, Wk, bk, Wv, bv, Wo):
    bf = ml_dtypes.bfloat16
    x2 = np.asarray(inputs, dtype=np.float32).reshape(NS, D)
    xt = np.ascontiguousarray(x2.T).astype(bf)
    cos_t, sin_t = _rope_tables()
    cos_b = cos_t.astype(bf)
    sin_b = sin_t.astype(bf)
    in_maps = []
    for c in range(NCORES):
        sl = slice(c * DPC, (c + 1) * DPC)
        in_maps.append(
            {
                "xt": xt,
                "wq": _prep_w(Wq[:, sl]),
                "wk": _prep_w(Wk[:, sl]),
                "wv": _prep_w(Wv[:, sl]),
                "wo": np.ascontiguousarray(Wo[sl, :]).astype(bf),
                "bq": np.ascontiguousarray(bq[sl]).reshape(DPC, 1).astype(np.float32),
                "bk": np.ascontiguousarray(bk[sl]).reshape(DPC, 1).astype(np.float32),
                "bv": np.ascontiguousarray(bv[sl]).reshape(1, DPC).astype(bf),
                "cos": cos_b,
                "sin": sin_b,
            }
        )
    return in_maps


def _get_nc():
    if "nc" not in _cache:
        _cache["nc"] = _build_nc()
    return _cache["nc"]


def run(inputs_dict, trace=False):
    """Build (cached), run on 8 cores, assemble full output. Returns
    (output fp32 [B,S,D], BassKernelResults)."""
    from concourse.bass_utils import run_bass_kernel_spmd

    nc = _get_nc()
    in_maps = _prep_inputs(
        inputs_dict["inputs"],
        inputs_dict["Wq"],
        inputs_dict["bq"],
        inputs_dict["Wk"],
        inputs_dict["bk"],
        inputs_dict["Wv"],
        inputs_dict["bv"],
        inputs_dict["Wo"],
    )
    res = run_bass_kernel_spmd(
        nc, in_maps, core_ids=list(range(NCORES)), trace=trace
    )
    acc = np.zeros((D, NS), dtype=np.float32)
    for r in res.results:
        acc += r["out"].astype(np.float32)
    out = acc.T.reshape(B, S, D) + np.asarray(inputs_dict["bo"], dtype=np.float32)
    return out.astype(np.float32), res


def kernel(**inputs):
    out, _ = run(inputs, trace=False)
    return out

